# revision 26
# baseline (speedup 1.0000x reference)
"""BitNet attention (GQA + RoPE) on 8 Trainium2 NeuronCores.

Tensor-parallel over heads: core c owns q-heads [4c, 4c+4), kv-head c.
Each core computes q/k/v projections (ternary BitNet weights), RoPE,
attention for its heads, and a row-parallel partial of the Wo
projection; the host sums the 8 partials.

v7 design (cost model measured on HW, see work/mb.py):
  - All matmuls bf16: moving operand streams 1 col/cyc @2.4GHz
    (216ns per 512 rows); row-tiled K=64 score pairs run concurrently
    (216ns/pair). Ternary weights exact in bf16.
  - exp on ACT, N=1024/call (1147ns sustained) from the PSUM score
    pair tile, bf16 out. ACT is the bottleneck engine: 128 calls =
    147us. Mask rides the activation bias (per-partition AP = per
    k-position), s_q*s_k/sqrt(d) rides the scale.
  - KV projections for all of S run first (DMA-overlapped), plus the
    j=0 Q projection; Q projections for j=1..3 and the Wo matmuls of
    block j-1 are emitted as background PE work, one or two items per
    attention chunk, so the exp stream starts ~25us in and stays hot.
  - Denominators via a ones-column in V (M=65 AV matmul); reciprocal
    batched as [128,8] (484ns vs 3.3us row-shaped); recip row is
    DMA-scattered/gathered via DRAM and broadcast on the sync queue.
  - PSUM: scores 2x[128,1024] (4 banks) + pA/pB (2) + Wo po (1) +
    bg q-proj accumulator (1) = 8 banks.
  - s_v*s_o applied in f32 during the po->bf16 output copy; output
    partials DMA'd as bf16 and summed on host.
"""

import sys

if "/opt/trn_rl_repo" not in sys.path:
    sys.path.insert(0, "/opt/trn_rl_repo")

import numpy as np

import concourse.bass as bass
from concourse import bacc, mybir
from concourse.bass import ts
from concourse.bass_utils import run_bass_kernel_spmd
from concourse.masks import make_identity
from concourse.tile import TileContext

F32 = mybir.dt.float32
F32R = mybir.dt.float32r
BF16 = mybir.dt.bfloat16

S = 2048
H = 2048
N_HEADS = 32
N_KV = 8
D = 64
NCORES = 8
HPC = N_HEADS // NCORES  # 4 q heads per core
OC = HPC * D  # 256 q output dims per core
NB = S // 512  # 4 s-blocks of 512
HC = H // 128  # 16 hidden chunks

LAST_EXEC_NS = None
LAST_TRACE = None
_CACHE = {}
DEBUG_DUMPS = False


def _ternarize(w):
    w = np.asarray(w, np.float32)
    s = (np.abs(w).mean() + np.float32(1e-6)).astype(np.float32)
    t = np.round(np.clip(w / s, np.float32(-1.0), np.float32(1.0))).astype(np.float32)
    return t, float(s)


def _build_program(s_qk, s_vo):
    nc = bacc.Bacc("TRN2", target_bir_lowering=False, debug=False, num_devices=NCORES)

    xt = nc.dram_tensor("xt", [NB, 128, HC, 512], BF16, kind="ExternalInput")
    wq = nc.dram_tensor("wq_t", [128, HC, OC], BF16, kind="ExternalInput")
    wkv = nc.dram_tensor("wkv_t", [128, HC, 128], BF16, kind="ExternalInput")
    wo = nc.dram_tensor("wo_t", [128, 2, H], BF16, kind="ExternalInput")
    cos_d = nc.dram_tensor("cos_t", [128, S], BF16, kind="ExternalInput")
    sin_d = nc.dram_tensor("sin_t", [128, S], BF16, kind="ExternalInput")
    mask_d = nc.dram_tensor("mask_t", [128, HC], F32, kind="ExternalInput")
    ones_d = nc.dram_tensor("ones_t", [128, HC], BF16, kind="ExternalInput")
    outp = nc.dram_tensor("outp", [S, H], BF16, kind="ExternalOutput")

    EXP = mybir.ActivationFunctionType.Exp
    MUL = mybir.AluOpType.mult
    ADD = mybir.AluOpType.add

    with TileContext(nc) as tc:
        with (
            tc.tile_pool(name="persist", bufs=1) as persist,
            tc.tile_pool(name="xtp", bufs=1) as xtp,
            tc.tile_pool(name="ph1t", bufs=3) as ph1t,
        ):
            qT = persist.tile([128, 2, S], BF16)
            kTd = persist.tile([128, S], BF16)
            V = persist.tile([128, HC, 65], BF16)
            aoT = persist.tile([128, 2, S], BF16)
            mask_sb = persist.tile([128, HC], F32)
            wo_sb = persist.tile([128, 2, H], BF16)
            wq_sb = persist.tile([128, HC, OC], BF16)
            wkv_sb = persist.tile([128, HC, 128], BF16)
            cos_sb = persist.tile([128, S], BF16)
            sin_sb = persist.tile([128, S], BF16)
            ident = persist.tile([64, 64], BF16)
            warm = persist.tile([1, 16], F32)

            # gpsimd queue: weights in use-order; wo last (needed ~80us in)
            nc.gpsimd.dma_start(wkv_sb[:, 0:8, :], wkv[:, 0:8, :])
            nc.gpsimd.dma_start(wq_sb[:, 0:8, :], wq[:, 0:8, :])
            nc.gpsimd.dma_start(wkv_sb[:, 8:16, :], wkv[:, 8:16, :])
            nc.gpsimd.dma_start(wq_sb[:, 8:16, :], wq[:, 8:16, :])
            nc.gpsimd.dma_start(cos_sb[:], cos_d[:])
            nc.gpsimd.dma_start(sin_sb[:], sin_d[:])
            nc.gpsimd.dma_start(mask_sb[:], mask_d[:])
            nc.gpsimd.dma_start(V[:, :, 64:65], ones_d[:])
            for k2 in range(2):
                nc.gpsimd.dma_start(wo_sb[:, k2, :], wo[:, k2, :])
            make_identity(nc, ident[:])
            nc.vector.memset(warm[:], 0.0)
            nc.scalar.activation(warm[:], warm[:], EXP, scale=1.0)

            xts = []

            def rope_q(p, j, qb):
                """emit rotate-half + cos/sin for one head pair; qb bf16 SBUF"""
                jb = ts(j, 512)
                rot = ph1t.tile([128, 512], BF16, tag="rot")
                nc.vector.tensor_copy(rot[0:32, :], qb[32:64, :])
                nc.vector.tensor_copy(rot[32:64, :], qb[0:32, :])
                nc.vector.tensor_copy(rot[64:96, :], qb[96:128, :])
                nc.vector.tensor_copy(rot[96:128, :], qb[64:96, :])
                qc = ph1t.tile([128, 512], BF16, tag="qc")
                nc.vector.tensor_tensor(qc[:], qb[:], cos_sb[:, jb], MUL)
                qs = ph1t.tile([128, 512], BF16, tag="qs")
                nc.vector.tensor_tensor(qs[:], rot[:], sin_sb[:, jb], MUL)
                nc.vector.tensor_tensor(qT[:, p, jb], qc[:], qs[:], ADD)

            # ---- Phase 1: KV projections for all blocks + Q for j=0 ----
            with (
                tc.tile_pool(name="ps1", bufs=2, space="PSUM") as ps1,
                tc.tile_pool(name="psvt", bufs=2, space="PSUM") as psvt,
            ):
                for b in range(NB):
                    sb = ts(b, 512)
                    xt_t = xtp.tile([128, HC, 512], BF16, tag=f"xt{b}")
                    xts.append(xt_t)
                    for hh in range(2):
                        nc.sync.dma_start(
                            xt_t[:, ts(hh, 8), :], xt[b, :, ts(hh, 8), :]
                        )
                    pkv = ps1.tile([128, 512], F32, tag="kv")
                    for c in range(HC):
                        nc.tensor.matmul(
                            pkv[:], wkv_sb[:, c, :], xt_t[:, c, :],
                            start=c == 0, stop=c == HC - 1,
                        )
                    if b == 0:
                        pq0 = ps1.tile([128, 512], F32, tag="q0")
                        pq1 = ps1.tile([128, 512], F32, tag="q1")
                        for c in range(HC):
                            st, sp = c == 0, c == HC - 1
                            nc.tensor.matmul(
                                pq0[:], wq_sb[:, c, 0:128], xt_t[:, c, :],
                                start=st, stop=sp,
                            )
                            nc.tensor.matmul(
                                pq1[:], wq_sb[:, c, 128:256], xt_t[:, c, :],
                                start=st, stop=sp,
                            )
                    # K: ACT copy + DVE RoPE (dup on both halves)
                    kb = ph1t.tile([64, 512], BF16, tag="kb")
                    nc.scalar.copy(kb[:], pkv[0:64, :])
                    vb = ph1t.tile([64, 512], BF16, tag="vb")
                    nc.vector.tensor_copy(vb[:], pkv[64:128, :])
                    pt = psvt.tile([128, 256], BF16, tag="vt")
                    for i4 in range(4):
                        nc.tensor.transpose(
                            pt[:, ts(i4, 64)], vb[:, ts(i4, 128)], ident[:]
                        )
                    nc.vector.tensor_copy(V[:, ts(b, 4), 0:64], pt[:])
                    rotk = ph1t.tile([64, 512], BF16, tag="rotk")
                    nc.vector.tensor_copy(rotk[0:32, :], kb[32:64, :])
                    nc.vector.tensor_copy(rotk[32:64, :], kb[0:32, :])
                    kc = ph1t.tile([64, 512], BF16, tag="kc")
                    nc.vector.tensor_tensor(kc[:], kb[:], cos_sb[0:64, sb], MUL)
                    ks = ph1t.tile([64, 512], BF16, tag="ks")
                    nc.vector.tensor_tensor(ks[:], rotk[:], sin_sb[0:64, sb], MUL)
                    nc.vector.tensor_tensor(kTd[0:64, sb], kc[:], ks[:], ADD)
                    nc.vector.tensor_tensor(kTd[64:128, sb], kc[:], ks[:], ADD)
                    if b == 0:
                        for p, pq in ((0, pq0), (1, pq1)):
                            qb = ph1t.tile([128, 512], BF16, tag="qb")
                            nc.scalar.copy(qb[:], pq[:])
                            rope_q(p, 0, qb)

            # ---- Phase 2: attention; bg queue feeds qproj(j+1) + Wo(j-1) ----
            from collections import deque

            bg = deque()

            with (
                tc.tile_pool(name="expp", bufs=4) as expp,
                tc.tile_pool(name="aoup", bufs=4) as aoup,
                tc.tile_pool(name="denp", bufs=2) as denp,
                tc.tile_pool(name="csd", bufs=4, space="DRAM") as csd,
                tc.tile_pool(name="bcp", bufs=4) as bcp,
                tc.tile_pool(name="osp", bufs=4) as osp,
                tc.tile_pool(name="pssc", bufs=2, space="PSUM") as pssc,
                tc.tile_pool(name="psav", bufs=1, space="PSUM") as psav,
                tc.tile_pool(name="pso", bufs=1, space="PSUM") as pso_,
                tc.tile_pool(name="psq", bufs=1, space="PSUM") as psq_,
            ):
                def arm_qproj(j):
                    """push bg items computing qT[:, :, j-block]"""
                    state = {}

                    def mk_mm(p, c):
                        def emit():
                            if c == 0:
                                state[p] = psq_.tile([128, 512], F32, tag="pq",
                                                     name=f"pq_{j}_{p}")
                            nc.tensor.matmul(
                                state[p][:],
                                wq_sb[:, c, ts(p, 128)], xts[j][:, c, :],
                                start=c == 0, stop=c == HC - 1,
                            )
                        return emit

                    def mk_fin(p):
                        def emit():
                            qb = ph1t.tile([128, 512], BF16, tag="qb")
                            nc.vector.tensor_copy(qb[:], state[p][:])
                            rope_q(p, j, qb)
                        return emit

                    for p in range(2):
                        for c in range(HC):
                            bg.append(mk_mm(p, c))
                        bg.append(mk_fin(p))

                def arm_wo(j, halves=(0, 1), last=False):
                    """push bg items computing outp rows for j-block j"""
                    state = {}

                    def mk(t, half):
                        jq = 4 * j + t // 4
                        hb = t % 4

                        def emit():
                            if half == 0:
                                state[t] = pso_.tile([128, 512], F32, tag="po",
                                                     name=f"po_{j}_{t}")
                                nc.tensor.matmul(
                                    state[t][:], aoT[:, 0, ts(jq, 128)],
                                    wo_sb[:, 0, ts(hb, 512)],
                                    start=True, stop=False,
                                )
                            else:
                                nc.tensor.matmul(
                                    state[t][:], aoT[:, 1, ts(jq, 128)],
                                    wo_sb[:, 1, ts(hb, 512)],
                                    start=False, stop=True,
                                )
                                ob = osp.tile([128, 512], BF16, tag="ob",
                                              name=f"ob_{j}_{t}")
                                if last:
                                    nc.scalar.mul(ob[:], state[t][:], s_vo)
                                else:
                                    nc.vector.tensor_scalar_mul(
                                        ob[:], state[t][:], s_vo
                                    )
                                nc.gpsimd.dma_start(
                                    outp[ts(jq, 128), ts(hb, 512)], ob[:]
                                )
                        return emit

                    for t in range(16):
                        for half in halves:
                            bg.append(mk(t, half))

                def bg_step(budget):
                    for _ in range(budget):
                        if bg:
                            bg.popleft()()

                for j in range(NB):
                    jb = ts(j, 512)
                    if j + 1 < NB:
                        arm_qproj(j + 1)
                    if j > 0:
                        arm_wo(j - 1)
                    for p in range(2):
                        chunks_left = (2 - p) * 16
                        pA = psav.tile([65, 512], F32, tag="avA")
                        pB = psav.tile([65, 512], F32, tag="avB")
                        for i in range(HC):
                            psS = pssc.tile([128, 1024], F32, tag="s")
                            nc.tensor.matmul(
                                psS[:, 0:512], kTd[0:64, ts(i, 128)],
                                qT[0:64, p, jb], start=True, stop=True,
                            )
                            nc.tensor.matmul(
                                psS[:, 512:1024], kTd[64:128, ts(i, 128)],
                                qT[64:128, p, jb], start=True, stop=True,
                            )
                            e2 = expp.tile([128, 1024], BF16, tag="e2",
                                           name=f"e2_{j}_{p}_{i}")
                            nc.scalar.activation(
                                e2[:], psS[:], EXP,
                                bias=mask_sb[:, i : i + 1], scale=s_qk,
                            )
                            st, sp = i == 0, i == HC - 1
                            nc.tensor.matmul(
                                pA[:], V[:, i, :], e2[:, 0:512], start=st, stop=sp
                            )
                            nc.tensor.matmul(
                                pB[:], V[:, i, :], e2[:, 512:1024], start=st, stop=sp
                            )
                            nbg = 2 if len(bg) > chunks_left - i else 1
                            bg_step(nbg)
                        # post-process: copy out, batched reciprocal, normalize
                        aoU = []
                        for h, pX in ((0, pA), (1, pB)):
                            u = aoup.tile([65, 512], F32, tag="aou",
                                          name=f"aou_{j}_{p}_{h}")
                            nc.vector.tensor_copy(u[:], pX[:])
                            aoU.append(u)
                        den = denp.tile([128, 8], F32, tag="den")
                        rden = denp.tile([128, 8], F32, tag="rden")
                        cs_dram = csd.tile([2, 1, 512], F32, tag="csd")
                        for h in range(2):
                            nc.sync.dma_start(den[:, ts(h, 4)], aoU[h][64:65, :])
                        nc.vector.reciprocal(rden[:], den[:])
                        for h in range(2):
                            nc.sync.dma_start(cs_dram[h], rden[:, ts(h, 4)])
                        for h in range(2):
                            bc = bcp.tile([64, 512], F32, tag="bc",
                                          name=f"bc_{j}_{p}_{h}")
                            nc.sync.dma_start(
                                bc[:], cs_dram[h].to_broadcast((64, 512))
                            )
                            nc.vector.tensor_tensor(
                                aoT[h * 64 : (h + 1) * 64, p, jb],
                                aoU[h][0:64, :], bc[:], MUL,
                            )
                # tail: Wo of the last block, output copies on ACT
                bg_step(len(bg))
                arm_wo(NB - 1, last=True)
                bg_step(len(bg))

    nc.compile()
    return nc


def kernel(
    hidden_states,
    attention_mask,
    position_ids,
    wq,
    wk,
    wv,
    wo,
    _trace=False,
):
    global LAST_EXEC_NS, LAST_TRACE
    import ml_dtypes

    x = np.asarray(hidden_states, np.float32)[0]  # [S, H]
    mask = np.asarray(attention_mask, np.float32)[0]  # [S]
    pos = np.asarray(position_ids)[0].astype(np.float32)  # [S]

    wq_t, s_q = _ternarize(wq)
    wk_t, s_k = _ternarize(wk)
    wv_t, s_v = _ternarize(wv)
    wo_t, s_o = _ternarize(wo)
    s_qk = float(np.float32(s_q) * np.float32(s_k) / np.float32(8.0))
    s_vo = float(np.float32(s_v) * np.float32(s_o))

    key = ("v7", s_qk, s_vo)
    if key not in _CACHE:
        _CACHE.clear()
        _CACHE[key] = _build_program(s_qk, s_vo)
    nc = _CACHE[key]

    bf = ml_dtypes.bfloat16
    xt_host = np.ascontiguousarray(
        x.T.reshape(HC, 128, NB, 512).transpose(2, 1, 0, 3)
    ).astype(bf)
    inv = (
        1.0 / (10000.0 ** (np.arange(0, D, 2, dtype=np.float32) / np.float32(D)))
    ).astype(np.float32)
    fr = pos[:, None] * inv[None, :]  # [S, 32]
    emb = np.concatenate([fr, fr], axis=1)  # [S, 64]
    cos64 = np.cos(emb).astype(np.float32)
    sin64 = np.sin(emb).astype(np.float32)
    sin64[:, : D // 2] *= -1.0
    cos128 = np.ascontiguousarray(np.vstack([cos64.T, cos64.T])).astype(bf)
    sin128 = np.ascontiguousarray(np.vstack([sin64.T, sin64.T])).astype(bf)
    mask_r = np.ascontiguousarray(mask.reshape(HC, 128).T)  # [128, HC]
    ones_r = np.ones((128, HC), dtype=bf)

    in_maps = []
    for c in range(NCORES):
        wq_c = np.ascontiguousarray(
            wq_t[c * OC : (c + 1) * OC, :].T.reshape(HC, 128, OC).transpose(1, 0, 2)
        ).astype(bf)
        wk_c = wk_t[c * D : (c + 1) * D, :].T  # [H, 64]
        wv_c = wv_t[c * D : (c + 1) * D, :].T
        wkv_c = np.ascontiguousarray(
            np.concatenate([wk_c, wv_c], axis=1).reshape(HC, 128, 128).transpose(1, 0, 2)
        ).astype(bf)
        wo_c = np.ascontiguousarray(
            wo_t[:, c * OC : (c + 1) * OC].T.reshape(2, 128, H).transpose(1, 0, 2)
        ).astype(bf)
        in_maps.append(
            {
                "xt": xt_host,
                "wq_t": wq_c,
                "wkv_t": wkv_c,
                "wo_t": wo_c,
                "cos_t": cos128,
                "sin_t": sin128,
                "mask_t": mask_r,
                "ones_t": ones_r,
            }
        )

    res = run_bass_kernel_spmd(
        nc, in_maps, core_ids=list(range(NCORES)), trace=bool(_trace)
    )
    LAST_EXEC_NS = res.exec_time_ns
    LAST_TRACE = res.instructions_and_trace[1] if res.instructions_and_trace else None

    out = res.results[0]["outp"].astype(np.float32)
    for c in range(1, NCORES):
        out = out + res.results[c]["outp"].astype(np.float32)
    return out.reshape(1, S, H).astype(np.float32)
